# revision 3
# baseline (speedup 1.0000x reference)
# Trainium2 Bass kernel v2 for EvidenceRetriever (cosine-sim retrieval + top-8).
#
# score[t, s] = <t_hat, s_hat> + 0.1 * importance[s]
# outputs: top-8 indices (int32), top-8 scores (f32, desc), softmax over the 8.
#
# Sharding: data-parallel over target rows - 8 cores x 2048 rows each;
# sources replicated. No cross-core communication.
#
# v2 changes vs v1:
#  - phase B: one gather per candidate from a prebuilt DRAM table
#    [E_S, 516] = [s_hat (512) | imp | pad]; fused DVE dot; no per-candidate
#    normalization (was: 2 gathers + ACT square + sqrt + recip per candidate).
#  - stage 2: candidate ids recovered by packing quantized score + local index
#    into one integer (2 wide DVE passes) instead of 16 is_equal scans.
#  - source normalize multiply moved to ACT (activation Copy with scale AP).
#  - deeper tile pools for cross-iteration pipelining.
import os
from contextlib import ExitStack

import numpy as np

import concourse.bass as bass
import concourse.tile as tile
from concourse import bacc, mybir
from concourse.bass_utils import run_bass_kernel_spmd
from concourse.masks import make_identity

F32 = mybir.dt.float32
F32R = mybir.dt.float32r
BF16 = mybir.dt.bfloat16
U32 = mybir.dt.uint32
I32 = mybir.dt.int32
AF = mybir.ActivationFunctionType
ALU = mybir.AluOpType

N_CORES = 8
E_T, E_S, FDIM = 16384, 32768, 512
T_LOCAL = E_T // N_CORES
K = 8
NCAND = 24
W_IMPORTANCE = 0.1
CHUNK = 1024
TBLW = 516            # table row: 512 feats + imp + 3 pad

LAST_RESULTS = None


def build_program(t_local=T_LOCAL, e_s=E_S, fdim=FDIM, chunk=CHUNK,
                  ablate=(), repeat=1):
    assert t_local % 128 == 0 and e_s % chunk == 0 and chunk % 512 == 0
    n_tt = t_local // 128
    n_sc = e_s // chunk
    n_kt = fdim // 128
    n_seg = chunk // 512
    assert n_sc % 2 == 0
    n_pair = n_sc // 2             # screening works on 2-chunk strips
    cchunk = 2 * chunk
    cw = n_pair * K                # compact width per target tile (128)
    ncand = min(NCAND, cw)

    nc = bacc.Bacc(None, target_bir_lowering=False, debug=False)
    t_in = nc.dram_tensor("t", [t_local, fdim], F32, kind="ExternalInput")
    s_in = nc.dram_tensor("s", [e_s, fdim], F32, kind="ExternalInput")
    imp_in = nc.dram_tensor("imp", [1, e_s], F32, kind="ExternalInput")
    impt_in = nc.dram_tensor("impt", [e_s, 1], F32, kind="ExternalInput")
    if repeat > 1:
        nc.dram_tensor(f"repdummy{repeat}", [1, repeat], F32,
                       kind="ExternalInput")
    idx_out = nc.dram_tensor("idx", [t_local, K], I32, kind="ExternalOutput")
    score_out = nc.dram_tensor("score", [t_local, K], F32,
                               kind="ExternalOutput")
    alpha_out = nc.dram_tensor("alpha", [t_local, K], F32,
                               kind="ExternalOutput")
    table = nc.dram_tensor("tbl", [e_s, TBLW], F32, kind="Internal")

    with tile.TileContext(nc) as tc:
        with ExitStack() as ctx:
            const = ctx.enter_context(tc.tile_pool(name="const", bufs=1))
            prep = ctx.enter_context(tc.tile_pool(name="prep", bufs=4))
            tpsum = ctx.enter_context(
                tc.tile_pool(name="tpsum", bufs=2, space="PSUM"))
            sT_pool = ctx.enter_context(tc.tile_pool(name="sT", bufs=4))
            mm_psum = ctx.enter_context(
                tc.tile_pool(name="mm", bufs=3, space="PSUM"))
            small = ctx.enter_context(tc.tile_pool(name="small", bufs=4))
            fin = ctx.enter_context(tc.tile_pool(name="fin", bufs=2))
            gat = ctx.enter_context(tc.tile_pool(name="gat", bufs=6))
            sbp = ctx.enter_context(tc.tile_pool(name="sbp", bufs=4))

            identity = const.tile([128, 128], F32)
            make_identity(nc, identity[:])
            iota_nc_i = const.tile([128, ncand], I32)
            nc.gpsimd.iota(iota_nc_i[:], pattern=[[1, ncand]], base=0,
                           channel_multiplier=0)
            iota_nc = const.tile([128, ncand], F32)
            nc.vector.tensor_copy(iota_nc[:], iota_nc_i[:])


            # Residents: transposed f32r targets, per-tile compact top-8
            # values (bf16) and local indices.
            tT_all = const.tile([128, n_kt * t_local], F32R)
            cvals = [const.tile([128, cw], BF16, name=f"cvals{i}")
                     for i in range(n_tt)]
            clidx = [const.tile([128, cw], U32, name=f"clidx{i}")
                     for i in range(n_tt)]

            def norm_rows(src_rows, norm, w):
                """DMA 128 rows + L2-normalize into norm[:, 0:fdim] (no PE)."""
                raw = prep.tile([128, fdim], F32, tag="raw")
                nc.sync.dma_start(raw[:], src_rows)
                sq = prep.tile([128, fdim], F32, tag="sq")
                ss = prep.tile([128, 1], F32, tag="ss")
                nc.scalar.activation(sq[:], raw[:], AF.Square, accum_out=ss[:])
                nrm = prep.tile([128, 1], F32, tag="nrm")
                nc.scalar.sqrt(nrm[:], ss[:])
                inv = prep.tile([128, 1], F32, tag="inv")
                nc.vector.reciprocal(inv[:], nrm[:])
                nc.scalar.activation(norm[:, 0:fdim], raw[:], AF.Copy,
                                     scale=inv[:])

            def transpose_into(dstT_all, dst_col, norm):
                """PE-transpose norm's 128x128 blocks into dstT_all slices."""
                pt = tpsum.tile([128, n_kt * 128], F32)
                for j in range(n_kt):
                    nc.tensor.transpose(
                        pt[:, j * 128:(j + 1) * 128],
                        norm[:, j * 128:(j + 1) * 128], identity[:])
                dst3 = dstT_all.rearrange(
                    "p (j c) -> p j c", j=n_kt)[:, :, dst_col:dst_col + 128]
                nc.scalar.copy(
                    dst3, pt[:].rearrange("p (j c) -> p j c", j=n_kt))

            # Phase 0: targets (feature 511 replaced by 1.0 so the source
            # side's slot 511 bias rides the 4th K-tile)
            for tt in range(n_tt):
                t0norm = prep.tile([128, fdim], F32, tag="tnorm")
                norm_rows(t_in.ap()[tt * 128:(tt + 1) * 128, :], t0norm, fdim)
                if "bias_mm" not in ablate:
                    nc.vector.memset(t0norm[:, fdim - 1:fdim], 1.0)
                transpose_into(tT_all, tt * 128, t0norm)

            # Source chunk prep is split into per-row-tile tasks that the
            # driver pumps between mm_screen calls, so next-chunk transposes
            # interleave into this chunk's PE stream instead of queuing
            # behind it.
            sT_cur = {}
            snorms = {}

            def t_norm(sc, r):
                row0 = sc * chunk + r * 128
                snorm = prep.tile([128, fdim + 1], F32, tag="snorm")
                norm_rows(s_in.ap()[row0:row0 + 128, :], snorm, fdim + 1)
                nc.sync.dma_start(snorm[:, fdim:fdim + 1],
                                  impt_in.ap()[row0:row0 + 128, :])
                snorms[(sc, r)] = snorm

            def t_tp(sc, r, rep0):
                if r == 0:
                    sT_cur[sc] = sT_pool.tile([128, n_kt * chunk], F32R,
                                              tag="sT", name="sTall")
                snorm = snorms.pop((sc, r))
                row0 = sc * chunk + r * 128
                if rep0:
                    nc.sync.dma_start(
                        table.ap()[row0:row0 + 128, 0:fdim + 1],
                        snorm[:, 0:fdim + 1])
                if "bias_mm" not in ablate:
                    # fold 0.1*imp into screening slot 511 (after the exact
                    # s_hat row was written to the table)
                    nc.scalar.activation(
                        snorm[:, fdim - 1:fdim], snorm[:, fdim:fdim + 1],
                        AF.Copy, scale=W_IMPORTANCE)
                transpose_into(sT_cur[sc], r * 128, snorm)

            def chunk_tasks(sc, rep0):
                for r in range(chunk // 128):
                    yield (lambda r=r: t_norm(sc, r))
                    yield (lambda r=r: t_tp(sc, r, rep0))

            from collections import deque
            prep_q = deque()

            def pump(n):
                for _ in range(n):
                    if prep_q:
                        prep_q.popleft()()

            # screening: ACT copies two PSUM strips to one SBUF bf16 2-chunk
            # strip (Max runs 4x on bf16); MaxIndex for strip i is emitted
            # during strip i+1 so it never stalls the DVE queue behind its
            # Max.
            pending = []

            def flush_screen():
                while pending:
                    cv, cl, sb = pending.pop(0)
                    if "maxidx" not in ablate:
                        nc.vector.max_index(cl, cv, sb[:])
                    else:
                        nc.vector.memset(cl, 0)

            def mm_only(tt, sc, sT_all):
                ps = mm_psum.tile([128, chunk], F32, tag="ps")
                for n in range(n_seg):
                    seg = ps[:, n * 512:(n + 1) * 512]
                    for j in range(n_kt):
                        nc.tensor.matmul(
                            seg,
                            tT_all[:, j * t_local + tt * 128:
                                   j * t_local + (tt + 1) * 128],
                            sT_all[:, j * chunk + n * 512:
                                   j * chunk + (n + 1) * 512],
                            start=(j == 0), stop=(j == n_kt - 1))
                return ps

            def mm_screen_pair(tt, p, sT_a, sT_b):
                sb = sbp.tile([128, cchunk], BF16, tag="sb")
                ps_a = mm_only(tt, 2 * p, sT_a)
                nc.scalar.copy(sb[:, 0:chunk], ps_a[:])
                ps_b = mm_only(tt, 2 * p + 1, sT_b)
                nc.scalar.copy(sb[:, chunk:cchunk], ps_b[:])
                cv = cvals[tt][:, p * K:(p + 1) * K]
                cl = clidx[tt][:, p * K:(p + 1) * K]
                if "max" not in ablate:
                    flush_screen()
                    nc.vector.max(cv, sb[:])
                    pending.append((cv, cl, sb))
                else:
                    nc.vector.memset(cv, 0.0)
                    nc.vector.memset(cl, 0)

            def stage2_phaseB(tt):
                flush_screen()
                # pack quantized value + local idx: ((v*4096)|>i32)*1024+lidx'
                qi = fin.tile([128, cw], I32, tag="qi")
                nc.vector.tensor_scalar(qi[:], cvals[tt][:], 4096.0, None,
                                        op0=ALU.mult)
                packed_i = fin.tile([128, cw], I32, tag="packedi")
                nc.vector.scalar_tensor_tensor(
                    packed_i[:], qi[:], float(cchunk), clidx[tt][:],
                    op0=ALU.mult, op1=ALU.add)
                packed_f = fin.tile([128, cw], F32, tag="packedf")
                nc.vector.tensor_copy(packed_f[:], packed_i[:])

                # top-ncand packed values + their compact positions
                pk16 = fin.tile([128, ncand], F32, tag="pk16")
                pos16 = fin.tile([128, ncand], U32, tag="pos16")
                nc.vector.max(pk16[:, 0:8], packed_f[:])
                nc.vector.max_index(pos16[:, 0:8], pk16[:, 0:8], packed_f[:])
                src = packed_f
                for rr in range(1, ncand // 8):
                    scratch = fin.tile([128, cw], F32, tag=f"scratch{rr}")
                    nc.vector.match_replace(
                        scratch[:], pk16[:, 8 * rr - 8:8 * rr], src[:], -1e30)
                    nc.vector.max(pk16[:, 8 * rr:8 * rr + 8], scratch[:])
                    nc.vector.max_index(pos16[:, 8 * rr:8 * rr + 8],
                                        pk16[:, 8 * rr:8 * rr + 8], scratch[:])
                    src = scratch

                # unpack (exact int ops): lidx = packed & (cchunk-1) ;
                # gid = (pos>>3)*cchunk + lidx
                pk_i = fin.tile([128, ncand], I32, tag="pki")
                nc.vector.tensor_copy(pk_i[:], pk16[:])
                li = fin.tile([128, ncand], I32, tag="li")
                nc.vector.tensor_scalar(li[:], pk_i[:], cchunk - 1, None,
                                        op0=ALU.bitwise_and)
                cbu = fin.tile([128, ncand], U32, tag="cbu")
                nc.vector.tensor_scalar(cbu[:], pos16[:], 3, None,
                                        op0=ALU.logical_shift_right)
                cb_i = fin.tile([128, ncand], I32, tag="cbi")
                nc.vector.tensor_copy(cb_i[:], cbu[:])
                gid_i = fin.tile([128, ncand], I32, tag="gidi")
                nc.vector.scalar_tensor_tensor(
                    gid_i[:], cb_i[:], float(cchunk), li[:],
                    op0=ALU.mult, op1=ALU.add)
                gid_f = fin.tile([128, ncand], F32, tag="gidf")
                nc.vector.tensor_copy(gid_f[:], gid_i[:])

                # Phase B: exact fp32 rescore via table gathers
                traw = fin.tile([128, fdim], F32, tag="traw")
                nc.sync.dma_start(traw[:],
                                  t_in.ap()[tt * 128:(tt + 1) * 128, :])
                tss = fin.tile([128, 1], F32, tag="tss")
                tsq = fin.tile([128, fdim], F32, tag="tsq")
                nc.scalar.activation(tsq[:], traw[:], AF.Square,
                                     accum_out=tss[:])
                tnr = fin.tile([128, 1], F32, tag="tnr")
                nc.scalar.sqrt(tnr[:], tss[:])
                tiv = fin.tile([128, 1], F32, tag="tiv")
                nc.vector.reciprocal(tiv[:], tnr[:])
                tnb = fin.tile([128, fdim], F32, tag="tnb")
                nc.scalar.activation(tnb[:], traw[:], AF.Copy, scale=tiv[:])

                exact = fin.tile([128, ncand], F32, tag="exact")
                if "phaseB" in ablate:
                    nc.vector.memset(exact[:], 0.0)
                for m in range(ncand) if "phaseB" not in ablate else []:
                    g = gat.tile([128, TBLW], F32, tag="g")
                    nc.gpsimd.indirect_dma_start(
                        out=g[:], out_offset=None,
                        in_=table.ap(),
                        in_offset=bass.IndirectOffsetOnAxis(
                            ap=gid_i[:, m:m + 1], axis=0),
                        bounds_check=e_s - 1, oob_is_err=False)
                    dotc = gat.tile([128, 1], F32, tag="dotc")
                    gj = gat.tile([128, fdim], F32, tag="gj")
                    nc.vector.scalar_tensor_tensor(
                        gj[:], g[:, 0:fdim], 1.0, tnb[:],
                        op0=ALU.mult, op1=ALU.mult, accum_out=dotc[:])
                    nc.vector.scalar_tensor_tensor(
                        exact[:, m:m + 1], g[:, fdim:fdim + 1],
                        W_IMPORTANCE, dotc[:], op0=ALU.mult, op1=ALU.add)

                # final top-8 on exact scores; ids via unique positions
                fvals = fin.tile([128, K], F32, tag="fvals")
                nc.vector.max(fvals[:], exact[:])
                fpos = fin.tile([128, K], U32, tag="fpos")
                nc.vector.max_index(fpos[:], fvals[:], exact[:])
                fposf = fin.tile([128, K], F32, tag="fposf")
                nc.vector.tensor_copy(fposf[:], fpos[:])
                gfin_f = fin.tile([128, K], F32, tag="gfinf")
                junk2 = fin.tile([128, ncand], F32, tag="junk2")
                for k in range(K):
                    nc.vector.scalar_tensor_tensor(
                        junk2[:], iota_nc[:], fposf[:, k:k + 1], gid_f[:],
                        op0=ALU.is_equal, op1=ALU.mult,
                        accum_out=gfin_f[:, k:k + 1])
                gfin_i = fin.tile([128, K], I32, tag="gfini")
                nc.vector.tensor_copy(gfin_i[:], gfin_f[:])
                # softmax over the 8
                e = fin.tile([128, K], F32, tag="e")
                sume = fin.tile([128, 1], F32, tag="sume")
                nc.scalar.activation(e[:], fvals[:], AF.Exp, accum_out=sume[:])
                rse = fin.tile([128, 1], F32, tag="rse")
                nc.vector.reciprocal(rse[:], sume[:])
                alpha_t = fin.tile([128, K], F32, tag="al")
                nc.vector.tensor_scalar_mul(alpha_t[:], e[:], rse[:])

                rows = slice(tt * 128, (tt + 1) * 128)
                nc.sync.dma_start(idx_out.ap()[rows, :], gfin_i[:])
                nc.sync.dma_start(score_out.ap()[rows, :], fvals[:])
                nc.sync.dma_start(alpha_out.ap()[rows, :], alpha_t[:])

            # Driver: chunk c+1's prep tasks are pumped between chunk c's
            # mm_screen calls (fine-grain engine interleave); the last two
            # chunks are staggered per target tile so phase B overlaps the
            # tail of phase A.
            assert 4 * (chunk // 128) == 2 * n_tt
            for _rep in range(repeat):
                rep0 = _rep == 0
                if _rep == 0:
                    for t in chunk_tasks(0, rep0):
                        t()
                    for t in chunk_tasks(1, rep0):
                        t()
                for p in range(n_pair - 1):
                    prep_q.extend(chunk_tasks(2 * p + 2, rep0))
                    prep_q.extend(chunk_tasks(2 * p + 3, rep0))
                    for tt in range(n_tt):
                        mm_screen_pair(tt, p, sT_cur[2 * p],
                                       sT_cur[2 * p + 1])
                        pump(2)
                if _rep < repeat - 1:
                    prep_q.extend(chunk_tasks(0, False))
                    prep_q.extend(chunk_tasks(1, False))
                pl = n_pair - 1
                for tt in range(n_tt):
                    mm_screen_pair(tt, pl, sT_cur[2 * pl], sT_cur[2 * pl + 1])
                    stage2_phaseB(tt)
                    pump(2)

    nc.compile()
    return nc


_COMPILED = None


def _get_compiled():
    global _COMPILED
    if _COMPILED is None:
        _COMPILED = build_program()
    return _COMPILED


def make_in_maps(t, s, imp):
    t = np.ascontiguousarray(np.asarray(t, dtype=np.float32))
    s = np.ascontiguousarray(np.asarray(s, dtype=np.float32))
    imp = np.ascontiguousarray(
        np.asarray(imp, dtype=np.float32).reshape(1, -1))
    assert t.shape == (E_T, FDIM) and s.shape == (E_S, FDIM)
    return [
        {"t": t[i * T_LOCAL:(i + 1) * T_LOCAL], "s": s, "imp": imp,
         "impt": imp.reshape(-1, 1)}
        for i in range(N_CORES)
    ]


def kernel(target_edge_feats, source_edge_feats, source_importance,
           topk=8, chunk_size=4096):
    global LAST_RESULTS
    assert int(topk) == K
    nc = _get_compiled()
    in_maps = make_in_maps(target_edge_feats, source_edge_feats,
                           source_importance)
    res = run_bass_kernel_spmd(
        nc, in_maps, list(range(N_CORES)),
        trace=bool(os.environ.get("BASS_TRACE")))
    LAST_RESULTS = res
    idx = np.concatenate(
        [res.results[i]["idx"] for i in range(N_CORES)], axis=0)
    score = np.concatenate(
        [res.results[i]["score"] for i in range(N_CORES)], axis=0)
    alpha = np.concatenate(
        [res.results[i]["alpha"] for i in range(N_CORES)], axis=0)
    return (idx.astype(np.int32), score.astype(np.float32),
            alpha.astype(np.float32))
